# revision 1
# baseline (speedup 1.0000x reference)
"""CNN-MRF loss (retrieval kNN) on 8 Trainium2 NeuronCores.

Reference: cosine-similarity argmax between all 96x96 content patches and
96x96 style patches (3x3xC=128 patches, d=1152), gather matched style
patches, fold (overlap-add), MSE against content features.

Sharding: content-patch axis N split 8 ways (12 grid rows / core), style
replicated.  Two-pass retrieval per core:
  coarse: bf16 similarity S (128 content x 384 style tiles) = sum of 9
     shifted matmuls (contraction = channels on partitions) accumulated
     in PSUM, scaled by replicated 1/||s||, stored bf16; DVE max8 +
     find_index8 give the top-8 candidate style patches per content patch.
     bf16 quantization (~3e-4) is far below the top-8 margin (~5e-2), so
     the true argmax is always among the 8.
  rescore: indirect-DMA gather of the 8 candidate patch rows (fp32),
     exact fp32 dot x 1/||s|| on DVE, one-hot select of the winner.
  then: indirect-DMA gather of the matched (un-normalized) style patch
     rows, PE transposes to channel-major, DVE fold accumulation into a
     14-row output strip.
Host: sums the 8 overlapping strips, divides by fold counts, MSE.
"""
import sys
import numpy as np

for _p in ("/opt/trn_rl_repo",):
    if _p not in sys.path:
        sys.path.insert(0, _p)

import concourse.bass as bass
import concourse.bacc as bacc
import concourse.mybir as mybir
from concourse.bass import IndirectOffsetOnAxis
from concourse.bass_utils import run_bass_kernel_spmd
from concourse.tile import TileContext
from concourse.masks import make_identity

F32 = mybir.dt.float32
BF16 = mybir.dt.bfloat16
U32 = mybir.dt.uint32

C = 128          # channels
H = W = 96       # feature-map spatial dims
PW = 3           # patch size
HP = H + 2       # padded spatial
N = H * W        # content patches total (9216)
M = N            # style patches (9216)
D = C * PW * PW  # patch vector length (1152)
NCORES = 8
RPC = H // NCORES       # content grid rows per core (12)
NSH = RPC * W           # content patches per core (1152)
NT = NSH // 128         # n-tiles of 128 per core (9)
MROWS = 4               # style grid rows per m-tile
MW = MROWS * W          # m-tile width (384)
MT = M // MW            # m-tiles (24)
import os
TOPK = 8
RL = int(os.environ.get("RL", "0"))  # 0=coarse, 1=full rescore, 2=dots-only
RESCORE = RL >= 1


def ts(i, size):
    return slice(i * size, (i + 1) * size)


def build_program():
    nc = bacc.Bacc()

    cpad_bf = nc.declare_dram_parameter(
        "cpad_bf", [C, RPC + 2, HP], BF16, isOutput=False
    )
    spad_bf = nc.declare_dram_parameter("spad_bf", [C, HP, HP], BF16, isOutput=False)
    sprows = nc.declare_dram_parameter("sprows", [M, D], F32, isOutput=False)
    cprows = nc.declare_dram_parameter("cprows", [NSH, D], F32, isOutput=False)
    invn_row = nc.declare_dram_parameter("invn_row", [1, M], F32, isOutput=False)
    idx_out = nc.declare_dram_parameter("idx_out", [NT, 128, 1], U32, isOutput=True)
    racc_out = nc.declare_dram_parameter(
        "racc_out", [C, RPC + 2, W], F32, isOutput=True
    )

    with TileContext(nc) as tc:
        with (
            tc.tile_pool(name="const", bufs=1) as constp,
            tc.tile_pool(name="big", bufs=1) as bigp,
            tc.tile_pool(name="work", bufs=2) as workp,
            tc.tile_pool(name="psS", bufs=4, space="PSUM") as psS,
            tc.tile_pool(name="psT", bufs=2, space="PSUM") as psT,
            tc.tile_pool(name="psN", bufs=2, space="PSUM") as psN,
        ):
            # ---- constants / loads ----
            ones_row = constp.tile([1, 128], F32)     # for partition broadcast
            nc.vector.memset(ones_row[:], 1.0)
            ident = constp.tile([128, 128], F32)
            make_identity(nc, ident[:])

            spad_t = bigp.tile([C, HP, HP], BF16)
            nc.sync.dma_start(out=spad_t[:], in_=spad_bf[:])
            cpad_t = bigp.tile([C, RPC + 2, HP], BF16)
            nc.sync.dma_start(out=cpad_t[:], in_=cpad_bf[:])

            # ---- style inverse norms, partition-broadcast: invb (128, M) ----
            invb = bigp.tile([C, M], F32)
            for t in range(MT):
                invn_t = workp.tile([1, MW], F32, tag="invn")
                nc.sync.dma_start(out=invn_t[:], in_=invn_row[0:1, ts(t, MW)])
                psum_b = psN.tile([128, MW], F32, tag="psb")
                nc.tensor.matmul(
                    out=psum_b[:],
                    lhsT=ones_row[:],
                    rhs=invn_t[:],
                    start=True,
                    stop=True,
                )
                nc.vector.tensor_copy(invb[:, ts(t, MW)], psum_b[:])

            # ---- contiguous shifted content views (bf16 weights) ----
            cshift = bigp.tile([C, 9, NSH], BF16)
            for k in range(9):
                ki, kj = k // 3, k % 3
                nc.vector.tensor_copy(
                    cshift[:, k], cpad_t[:, ki : ki + RPC, kj : kj + W]
                )

            # ---- coarse similarity + top-8 + rescore + gather + fold ----
            racc = bigp.tile([C, RPC + 2, HP], F32)
            nc.gpsimd.memset(racc[:], 0.0)

            MTILES = [(5 * i, 5) for i in range(19)] + [(95, 1)]
            for j in range(NT):
                S_sb = bigp.tile([C, M], BF16, tag="S_sb", bufs=2)
                cprows_j = workp.tile([128, D], F32, tag="cpr")
                nc.sync.dma_start(out=cprows_j[:], in_=cprows[ts(j, 128), :])

                for g in range(0, len(MTILES), 4):
                    grp = []
                    for (mrow, nr) in MTILES[g : g + 4]:
                        pt = psS.tile([128, 480], F32, tag="psS", name=f"ps_{j}_{g}_{mrow}")
                        grp.append((pt, mrow, nr))
                    for k in range(9):
                        ki, kj = k // 3, k % 3
                        lhsT = cshift[:, k, ts(j, 128)]
                        for (pt, mrow, nr) in grp:
                            nc.tensor.matmul(
                                out=pt[:, : nr * W],
                                lhsT=lhsT,
                                rhs=spad_t[
                                    :, mrow + ki : mrow + ki + nr, kj : kj + W
                                ],
                                start=(k == 0),
                                stop=(k == 8),
                            )
                    for (pt, mrow, nr) in grp:
                        nc.vector.tensor_mul(
                            S_sb[:, mrow * W : (mrow + nr) * W],
                            pt[:, : nr * W],
                            invb[:, mrow * W : (mrow + nr) * W],
                        )
                max8 = workp.tile([128, 8], BF16, tag="max8")
                nc.vector.max(max8[:], S_sb[:])
                idx8 = workp.tile([128, 8], U32, tag="idx8")
                nc.vector.max_index(idx8[:], max8[:], S_sb[:])

                if RESCORE:
                    # ---- exact fp32 rescore of the 8 candidates ----
                    use_rescore = RESCORE
                    dots8 = workp.tile([128, 8], F32, tag="dots8")
                    nrm28 = workp.tile([128, 8], F32, tag="nrm28")
                    for cc in range(TOPK):
                        idxcc = workp.tile([128, 1], U32, tag="idxcc")
                        nc.vector.tensor_copy(idxcc[:], idx8[:, cc : cc + 1])
                        gath = workp.tile([128, D], F32, tag="gath")
                        nc.gpsimd.indirect_dma_start(
                            out=gath[:],
                            out_offset=None,
                            in_=sprows[:],
                            in_offset=IndirectOffsetOnAxis(
                                ap=idxcc[:, 0:1], axis=0
                            ),
                        )
                        scr = workp.tile([128, D], F32, tag="scr")
                        scr2 = workp.tile([128, D], F32, tag="scr2")
                        nc.vector.tensor_tensor_reduce(
                            out=scr[:],
                            in0=gath[:],
                            in1=cprows_j[:],
                            scale=1.0,
                            scalar=0.0,
                            op0=mybir.AluOpType.mult,
                            op1=mybir.AluOpType.add,
                            accum_out=dots8[:, cc : cc + 1],
                        )
                        nc.vector.tensor_tensor_reduce(
                            out=scr2[:],
                            in0=gath[:],
                            in1=gath[:],
                            scale=1.0,
                            scalar=0.0,
                            op0=mybir.AluOpType.mult,
                            op1=mybir.AluOpType.add,
                            accum_out=nrm28[:, cc : cc + 1],
                        )
                    sq8 = workp.tile([128, 8], F32, tag="sq8")
                    nc.scalar.activation(
                        sq8[:], nrm28[:], mybir.ActivationFunctionType.Sqrt
                    )
                    if RL == 2:
                        bestu = workp.tile([128, 1], U32, tag="bestu")
                        nc.vector.tensor_copy(bestu[:], idx8[:, 0:1])
                        nc.sync.dma_start(out=idx_out[j], in_=bestu[:])
                    if RL == 1:
                        rec8 = workp.tile([128, 8], F32, tag="rec8")
                        nc.vector.reciprocal(rec8[:], sq8[:])
                        s8 = workp.tile([128, 8], F32, tag="s8")
                        nc.vector.tensor_mul(s8[:], dots8[:], rec8[:])
                        top8 = workp.tile([128, 8], F32, tag="top8")
                        nc.vector.max(top8[:], s8[:])
                        onehot = workp.tile([128, 8], F32, tag="onehot")
                        nc.vector.tensor_tensor(
                            out=onehot[:],
                            in0=s8[:],
                            in1=top8[:, 0:1].to_broadcast((128, 8)),
                            op=mybir.AluOpType.is_equal,
                        )
                        idx8f = workp.tile([128, 8], F32, tag="idx8f")
                        nc.vector.tensor_copy(idx8f[:], idx8[:])
                        selscr = workp.tile([128, 8], F32, tag="selscr")
                        bestf = workp.tile([128, 1], F32, tag="bestf")
                        nc.vector.tensor_tensor_reduce(
                            out=selscr[:],
                            in0=onehot[:],
                            in1=idx8f[:],
                            scale=1.0,
                            scalar=-1.0,
                            op0=mybir.AluOpType.mult,
                            op1=mybir.AluOpType.max,
                            accum_out=bestf[:],
                        )
                        bestu = workp.tile([128, 1], U32, tag="bestu")
                        nc.vector.tensor_copy(bestu[:], bestf[:])
                        nc.sync.dma_start(out=idx_out[j], in_=bestu[:])
                else:
                    bestu = workp.tile([128, 1], U32, tag="bestu")
                    nc.vector.tensor_copy(bestu[:], idx8[:, 0:1])
                    nc.sync.dma_start(out=idx_out[j], in_=bestu[:])

                # gather matched style patch rows (n-major); the indirect
                # DMA needs a flat 2D dest (3D dest tiles fetch garbage)
                matched = workp.tile([128, D], F32, tag="matched")
                nc.gpsimd.indirect_dma_start(
                    out=matched[:],
                    out_offset=None,
                    in_=sprows[:],
                    in_offset=IndirectOffsetOnAxis(ap=bestu[:, 0:1], axis=0),
                )
                matched3 = matched[:].rearrange("p (a b) -> p a b", b=9)

                # transpose to channel-major and fold-accumulate
                n0 = j * 128
                r0, c0 = n0 // W, n0 % W
                seg1 = (r0, c0, W - c0, 0)
                seg2 = (r0 + 1, 0, 128 - (W - c0), W - c0)
                for k in range(9):
                    ki, kj = k // 3, k % 3
                    psum_T = psT.tile([128, 128], F32, tag="psT")
                    nc.tensor.transpose(psum_T[:], matched3[:, :, k], ident[:])
                    for (r, c, ln, off) in (seg1, seg2):
                        nc.vector.tensor_add(
                            racc[:, r + ki, c + kj : c + kj + ln],
                            racc[:, r + ki, c + kj : c + kj + ln],
                            psum_T[:, off : off + ln],
                        )

            nc.sync.dma_start(out=racc_out[:], in_=racc[:, :, 1 : 1 + W])

    if not nc.is_finalized():
        nc.finalize()
    return nc


_PROGRAM = None


def _get_program():
    global _PROGRAM
    if _PROGRAM is None:
        _PROGRAM = build_program()
    return _PROGRAM


def _patch_rows(x):
    """(C, R, Cc) padded map -> ((R-2)*(Cc-2), C*9) patch rows, (c,ki,kj)."""
    w = np.lib.stride_tricks.sliding_window_view(x, (PW, PW), axis=(1, 2))
    return np.ascontiguousarray(
        w.transpose(1, 2, 0, 3, 4).reshape((x.shape[1] - 2) * (x.shape[2] - 2), -1)
    )


def _host_prep(content_feats, style_feats):
    """Build per-core input maps."""
    bf = mybir.dt.np(BF16)
    cf = np.ascontiguousarray(np.asarray(content_feats, dtype=np.float32)[0])
    sf = np.ascontiguousarray(np.asarray(style_feats, dtype=np.float32)[0])
    cpad = np.pad(cf, ((0, 0), (1, 1), (1, 1)))
    spad = np.pad(sf, ((0, 0), (1, 1), (1, 1)))
    sprows = _patch_rows(spad)
    spad_b = spad.astype(bf)
    invn = (
        1.0
        / np.maximum(np.linalg.norm(sprows, axis=1), np.float32(1e-12))
    ).astype(np.float32)
    in_maps = []
    for i in range(NCORES):
        cslab = np.ascontiguousarray(cpad[:, i * RPC : i * RPC + RPC + 2, :])
        in_maps.append(
            {
                "cpad_bf": cslab.astype(bf),
                "spad_bf": spad_b,
                "sprows": sprows,
                "cprows": _patch_rows(cslab),
                "invn_row": np.ascontiguousarray(invn.reshape(1, M)),
            }
        )
    return cf, in_maps


_DIVISOR = None


def _fold_divisor():
    global _DIVISOR
    if _DIVISOR is None:
        cnt = np.full(H, 3, dtype=np.float32)
        cnt[0] = cnt[-1] = 2
        _DIVISOR = np.outer(cnt, cnt).astype(np.float32) + np.float32(1e-8)
    return _DIVISOR


def _host_combine(cf, results):
    acc = np.zeros((C, H + 2, W), dtype=np.float32)
    for i in range(NCORES):
        acc[:, i * RPC : i * RPC + RPC + 2, :] += results[i]["racc_out"]
    recon = acc[:, 1 : 1 + H, :] / _fold_divisor()[None, :, :]
    diff = cf - recon
    return np.float32(np.mean(np.square(diff), dtype=np.float64))


def run(content_feats, style_feats, trace=False):
    nc = _get_program()
    cf, in_maps = _host_prep(content_feats, style_feats)
    res = run_bass_kernel_spmd(
        nc, in_maps, core_ids=list(range(NCORES)), trace=trace
    )
    mse = _host_combine(cf, res.results)
    return mse, res


def kernel(content_feats, style_feats):
    mse, _ = run(content_feats, style_feats)
    return np.array(mse, dtype=np.float32)



# revision 9
# speedup vs baseline: 1.0734x; 1.0734x over previous
"""CNN-MRF loss (retrieval kNN) on 8 Trainium2 NeuronCores.

Reference: cosine-similarity argmax between all 96x96 content patches and
96x96 style patches (3x3xC=128 patches, d=1152), gather matched style
patches, fold (overlap-add), MSE against content features.

Sharding: content-patch axis N split 8 ways (12 grid rows / core), style
replicated.  Per core, per 128-patch tile j:
  coarse: bf16 similarity S (128 content x 9216 style) = sum of 9
     shifted matmuls (contraction = channels on partitions) accumulated
     in PSUM; fused DVE multiply-by-1/||s||-and-group-max produces S (bf16)
     plus 20 per-group maxima in one pass; vector.max over the group maxima
     + one max_index scan give the argmax style patch per content patch.
     bf16 rounding is monotonic, so max(round(x)) == round(max(x)) and the
     group-max value matches the stored bf16 S exactly.
  gather: indirect-DMA of the matched (un-normalized) style patch rows,
     PE transposes to channel-major, DVE fold accumulation into a
     14-row output strip.
The gather/transpose/fold of tile j is deferred until after tile j+1's
matmuls are issued so the PE queue never stalls on the DVE argmax chain.
Host: sums the 8 overlapping strips, divides by fold counts, MSE.
"""
import sys
import numpy as np

for _p in ("/opt/trn_rl_repo",):
    if _p not in sys.path:
        sys.path.insert(0, _p)

import concourse.bass as bass
import concourse.bacc as bacc
import concourse.mybir as mybir
from concourse.bass import IndirectOffsetOnAxis
from concourse.bass_utils import run_bass_kernel_spmd
from concourse.tile import TileContext
from concourse.masks import make_identity

F32 = mybir.dt.float32
BF16 = mybir.dt.bfloat16
U32 = mybir.dt.uint32

C = 128          # channels
H = W = 96       # feature-map spatial dims
PW = 3           # patch size
HP = H + 2       # padded spatial
N = H * W        # content patches total (9216)
M = N            # style patches (9216)
D = C * PW * PW  # patch vector length (1152)
NCORES = 8
RPC = H // NCORES       # content grid rows per core (12)
NSH = RPC * W           # content patches per core (1152)
NT = NSH // 128         # n-tiles of 128 per core (9)
MROWS = 5               # style grid rows per m-tile
MW = MROWS * W          # m-tile width (480)
MTILES = [(5 * i, 5) for i in range(19)] + [(95, 1)]  # (row0, nrows)
NG = len(MTILES)        # 20 matmul groups per j-tile


def ts(i, size):
    return slice(i * size, (i + 1) * size)


def build_program():
    nc = bacc.Bacc()

    cpad_bf = nc.declare_dram_parameter(
        "cpad_bf", [C, RPC + 2, HP], BF16, isOutput=False
    )
    spad_bf = nc.declare_dram_parameter("spad_bf", [C, HP, HP], BF16, isOutput=False)
    sprows = nc.declare_dram_parameter("sprows", [M, D], F32, isOutput=False)
    invn_row = nc.declare_dram_parameter("invn_row", [1, M], F32, isOutput=False)
    idx_out = nc.declare_dram_parameter("idx_out", [NT, 128, 1], U32, isOutput=True)
    racc_out = nc.declare_dram_parameter(
        "racc_out", [C, RPC + 2, W], F32, isOutput=True
    )

    with TileContext(nc) as tc:
        with (
            tc.tile_pool(name="const", bufs=1) as constp,
            tc.tile_pool(name="big", bufs=1) as bigp,
            tc.tile_pool(name="work", bufs=2) as workp,
            tc.tile_pool(name="psS", bufs=4, space="PSUM") as psS,
            tc.tile_pool(name="psT", bufs=1, space="PSUM") as psT,
        ):
            # ---- constants / loads ----
            ident = constp.tile([128, 128], F32)
            make_identity(nc, ident[:])

            cpad_t = bigp.tile([C, RPC + 2, HP], BF16)
            nc.sync.dma_start(out=cpad_t[:], in_=cpad_bf[:])
            spad_t = bigp.tile([C, HP, HP], BF16)
            nc.sync.dma_start(out=spad_t[:], in_=spad_bf[:])
            invn_sb = constp.tile([1, M], F32)
            nc.sync.dma_start(out=invn_sb[:], in_=invn_row[:])

            # style inverse norms broadcast to all partitions: invb (128, M)
            import os
            PBCAST = os.environ.get("PBCAST", "0") == "1"
            invb = bigp.tile([128, M], F32)
            if PBCAST:
                nc.gpsimd.partition_broadcast(invb[:], invn_sb[:])
            else:
                ones_row = constp.tile([1, 128], F32)
                nc.vector.memset(ones_row[:], 1.0)
                for t in range(M // 384):
                    psum_b = psS.tile([128, 480], F32, tag="psS", name=f"bc_{t}")
                    nc.tensor.matmul(
                        out=psum_b[:, :384],
                        lhsT=ones_row[:],
                        rhs=invn_sb[0:1, ts(t, 384)],
                        start=True,
                        stop=True,
                    )
                    nc.vector.tensor_copy(invb[:, ts(t, 384)], psum_b[:, :384])

            # ---- contiguous shifted content views (bf16 weights) ----
            cshift = bigp.tile([C, 9, NSH], BF16)
            for k in range(9):
                ki, kj = k // 3, k % 3
                nc.vector.tensor_copy(
                    cshift[:, k], cpad_t[:, ki : ki + RPC, kj : kj + W]
                )

            racc = bigp.tile([C, RPC + 2, HP], F32)
            nc.gpsimd.memset(racc[:], 0.0)

            FOLD1 = os.environ.get("FOLD1", "1") == "1"

            def fold(j, matched):
                """Transpose matched patches to channel-major and overlap-add."""
                matched3 = matched[:].rearrange("p (a b) -> p a b", b=9)
                n0 = j * 128
                r0, c0 = n0 // W, n0 % W
                seg1 = (r0, c0, W - c0, 0)
                seg2 = (r0 + 1, 0, 128 - (W - c0), W - c0)
                if FOLD1:
                    trT = psT.tile([128, 9, 128], F32, tag="psT", name=f"trT_{j}")
                    for k in range(9):
                        nc.tensor.transpose(trT[:, k], matched3[:, :, k], ident[:])
                    for k in range(9):
                        ki, kj = k // 3, k % 3
                        for (r, c, ln, off) in (seg1, seg2):
                            nc.vector.tensor_add(
                                racc[:, r + ki, c + kj : c + kj + ln],
                                racc[:, r + ki, c + kj : c + kj + ln],
                                trT[:, k, off : off + ln],
                            )
                else:
                    for k in range(9):
                        ki, kj = k // 3, k % 3
                        psum_T = psT.tile([128, 128], F32, tag="psT2", bufs=2)
                        nc.tensor.transpose(psum_T[:], matched3[:, :, k], ident[:])
                        for (r, c, ln, off) in (seg1, seg2):
                            nc.vector.tensor_add(
                                racc[:, r + ki, c + kj : c + kj + ln],
                                racc[:, r + ki, c + kj : c + kj + ln],
                                psum_T[:, off : off + ln],
                            )

            TTR = os.environ.get("TTR", "1") == "1"
            deferred = None
            for j in range(NT):
                S_sb = bigp.tile([128, M], BF16, tag="S_sb", bufs=2)
                gmax = workp.tile([128, NG], F32, tag="gmax")

                for g in range(0, NG, 4):
                    grp = []
                    for gi, (mrow, nr) in enumerate(MTILES[g : g + 4]):
                        pt = psS.tile(
                            [128, 480], F32, tag="psS", name=f"ps_{j}_{g + gi}"
                        )
                        grp.append((g + gi, pt, mrow, nr))
                    for k in range(9):
                        ki, kj = k // 3, k % 3
                        lhsT = cshift[:, k, ts(j, 128)]
                        for (_, pt, mrow, nr) in grp:
                            nc.tensor.matmul(
                                out=pt[:, : nr * W],
                                lhsT=lhsT,
                                rhs=spad_t[
                                    :, mrow + ki : mrow + ki + nr, kj : kj + W
                                ],
                                start=(k == 0),
                                stop=(k == 8),
                            )
                    for (gg, pt, mrow, nr) in grp:
                        if TTR:
                            # S = psum * invb, fused with per-group max
                            nc.vector.tensor_tensor_reduce(
                                out=S_sb[:, mrow * W : (mrow + nr) * W],
                                in0=pt[:, : nr * W],
                                in1=invb[:, mrow * W : (mrow + nr) * W],
                                scale=1.0,
                                scalar=-3.0e38,
                                op0=mybir.AluOpType.mult,
                                op1=mybir.AluOpType.max,
                                accum_out=gmax[:, gg : gg + 1],
                            )
                        else:
                            nc.vector.tensor_mul(
                                S_sb[:, mrow * W : (mrow + nr) * W],
                                pt[:, : nr * W],
                                invb[:, mrow * W : (mrow + nr) * W],
                            )

                if TTR:
                    # fp32 group maxima -> top8 -> round to bf16 (monotonic,
                    # so the rounded max matches the bf16 values stored in S)
                    vm8f = workp.tile([128, 8], F32, tag="vm8f")
                    nc.vector.max(vm8f[:], gmax[:])
                    vm8 = workp.tile([128, 8], BF16, tag="vm8")
                    nc.vector.tensor_copy(vm8[:], vm8f[:])
                else:
                    vm8 = workp.tile([128, 8], BF16, tag="vm8")
                    nc.vector.max(vm8[:], S_sb[:])
                idx8 = workp.tile([128, 8], U32, tag="idx8")
                nc.vector.max_index(idx8[:], vm8[:], S_sb[:])
                bestu = workp.tile([128, 1], U32, tag="bestu")
                nc.vector.tensor_copy(bestu[:], idx8[:, 0:1])
                nc.sync.dma_start(out=idx_out[j], in_=bestu[:])

                # gather matched style patch rows (n-major); the indirect
                # DMA needs a flat 2D dest (3D dest tiles fetch garbage)
                matched = workp.tile([128, D], F32, tag="matched")
                nc.gpsimd.indirect_dma_start(
                    out=matched[:],
                    out_offset=None,
                    in_=sprows[:],
                    in_offset=IndirectOffsetOnAxis(ap=bestu[:, 0:1], axis=0),
                )

                # fold of the previous tile, deferred so tile j+1's matmuls
                # are already queued on the PE before these transposes
                if deferred is not None:
                    fold(*deferred)
                deferred = (j, matched)

            fold(*deferred)
            nc.sync.dma_start(out=racc_out[:], in_=racc[:, :, 1 : 1 + W])

    if not nc.is_finalized():
        nc.finalize()
    return nc


_PROGRAM = None


def _get_program():
    global _PROGRAM
    if _PROGRAM is None:
        _PROGRAM = build_program()
    return _PROGRAM


def _patch_rows(x):
    """(C, R, Cc) padded map -> ((R-2)*(Cc-2), C*9) patch rows, (c,ki,kj)."""
    w = np.lib.stride_tricks.sliding_window_view(x, (PW, PW), axis=(1, 2))
    return np.ascontiguousarray(
        w.transpose(1, 2, 0, 3, 4).reshape((x.shape[1] - 2) * (x.shape[2] - 2), -1)
    )


def _host_prep(content_feats, style_feats):
    """Build per-core input maps."""
    bf = mybir.dt.np(BF16)
    cf = np.ascontiguousarray(np.asarray(content_feats, dtype=np.float32)[0])
    sf = np.ascontiguousarray(np.asarray(style_feats, dtype=np.float32)[0])
    cpad = np.pad(cf, ((0, 0), (1, 1), (1, 1)))
    spad = np.pad(sf, ((0, 0), (1, 1), (1, 1)))
    sprows = _patch_rows(spad)
    spad_b = spad.astype(bf)
    invn = (
        1.0
        / np.maximum(np.linalg.norm(sprows, axis=1), np.float32(1e-12))
    ).astype(np.float32)
    in_maps = []
    for i in range(NCORES):
        cslab = np.ascontiguousarray(cpad[:, i * RPC : i * RPC + RPC + 2, :])
        in_maps.append(
            {
                "cpad_bf": cslab.astype(bf),
                "spad_bf": spad_b,
                "sprows": sprows,
                "invn_row": np.ascontiguousarray(invn.reshape(1, M)),
            }
        )
    return cf, in_maps


_DIVISOR = None


def _fold_divisor():
    global _DIVISOR
    if _DIVISOR is None:
        cnt = np.full(H, 3, dtype=np.float32)
        cnt[0] = cnt[-1] = 2
        _DIVISOR = np.outer(cnt, cnt).astype(np.float32) + np.float32(1e-8)
    return _DIVISOR


def _host_combine(cf, results):
    acc = np.zeros((C, H + 2, W), dtype=np.float32)
    for i in range(NCORES):
        acc[:, i * RPC : i * RPC + RPC + 2, :] += results[i]["racc_out"]
    recon = acc[:, 1 : 1 + H, :] / _fold_divisor()[None, :, :]
    diff = cf - recon
    return np.float32(np.mean(np.square(diff), dtype=np.float64))


def run(content_feats, style_feats, trace=False):
    nc = _get_program()
    cf, in_maps = _host_prep(content_feats, style_feats)
    res = run_bass_kernel_spmd(
        nc, in_maps, core_ids=list(range(NCORES)), trace=trace
    )
    mse = _host_combine(cf, res.results)
    return mse, res


def kernel(content_feats, style_feats):
    mse, _ = run(content_feats, style_feats)
    return np.array(mse, dtype=np.float32)


# revision 10
# speedup vs baseline: 2.0407x; 1.9012x over previous
"""CNN-MRF loss (retrieval kNN) on 8 Trainium2 NeuronCores.

Reference: cosine-similarity argmax between all 96x96 content patches and
96x96 style patches (3x3xC=128 patches, d=1152), gather matched style
patches, fold (overlap-add), MSE against content features.

Sharding: content-patch axis N split 8 ways (12 grid rows / core), style
replicated.  Per core, per 128-patch tile j:
  similarity: fp8(e4m3) matmul of content patch rows against
     HOST-PRE-NORMALIZED style patch rows (style side absorbs 1/||s||, so
     no on-device scaling pass is needed).  Contraction over d=1152 runs
     as 5 DoubleRow passes (2x fp8 rate, 256-deep each, zero-padded to 10
     chunks of 128).  PSUM -> SBUF (bf16) copies run on the otherwise-idle
     Scalar engine, decoupling PSUM drain from the DVE argmax chain.
  argmax: DVE tensor_reduce(max) over S + one max_index scan.
     fp8 scoring moves the argmax for ~6% of patches to a near-equal
     neighbour; measured end-MSE error 1.3e-4, far inside tolerance.
  gather: indirect-DMA of the matched (un-normalized fp32) style patch
     rows, PE transposes to channel-major, DVE fold accumulation into a
     14-row output strip.  Deferred one iteration so the PE queue never
     waits on the argmax chain.
Host: sums the 8 overlapping strips, divides by fold counts, MSE.
"""
import sys
import numpy as np

for _p in ("/opt/trn_rl_repo",):
    if _p not in sys.path:
        sys.path.insert(0, _p)

import concourse.bass as bass
import concourse.bacc as bacc
import concourse.mybir as mybir
from concourse.bass import IndirectOffsetOnAxis
from concourse.bass_utils import run_bass_kernel_spmd
from concourse.tile import TileContext
from concourse.masks import make_identity

F32 = mybir.dt.float32
BF16 = mybir.dt.bfloat16
FP8 = mybir.dt.float8e4
U32 = mybir.dt.uint32

C = 128          # channels
H = W = 96       # feature-map spatial dims
PW = 3           # patch size
N = H * W        # content patches total (9216)
M = N            # style patches (9216)
D = C * PW * PW  # patch vector length (1152)
NCORES = 8
RPC = H // NCORES       # content grid rows per core (12)
NSH = RPC * W           # content patches per core (1152)
NT = NSH // 128         # n-tiles of 128 per core (9)
MG = 512                # style patches per matmul group
NG = M // MG            # matmul groups (18)
KC = 10                 # contraction chunks of 128 (9 real + 1 zero pad)


def ts(i, size):
    return slice(i * size, (i + 1) * size)


def build_program():
    nc = bacc.Bacc()

    cvT8 = nc.declare_dram_parameter("cvT8", [128, KC, NSH], FP8, isOutput=False)
    svnT8 = nc.declare_dram_parameter("svnT8", [NG, 128, KC, MG], FP8, isOutput=False)
    sprows = nc.declare_dram_parameter("sprows", [M, D], F32, isOutput=False)
    idx_out = nc.declare_dram_parameter("idx_out", [NT, 128, 1], U32, isOutput=True)
    racc_out = nc.declare_dram_parameter(
        "racc_out", [C, RPC + 2, W], F32, isOutput=True
    )

    with TileContext(nc) as tc:
        with (
            tc.tile_pool(name="const", bufs=1) as constp,
            tc.tile_pool(name="big", bufs=1) as bigp,
            tc.tile_pool(name="work", bufs=2) as workp,
            tc.tile_pool(name="psS", bufs=4, space="PSUM") as psS,
            tc.tile_pool(name="psT", bufs=1, space="PSUM") as psT,
        ):
            # ---- constants / loads ----
            ident = constp.tile([128, 128], F32)
            make_identity(nc, ident[:])

            cvT_sb = bigp.tile([128, KC, NSH], FP8)
            svn_sb = bigp.tile([128, NG, KC, MG], FP8)
            # first group + content rows first so matmuls can start early
            nc.sync.dma_start(out=svn_sb[:, 0], in_=svnT8[0])
            nc.sync.dma_start(out=cvT_sb[:], in_=cvT8[:])
            for g in range(1, NG):
                nc.sync.dma_start(out=svn_sb[:, g], in_=svnT8[g])

            racc = bigp.tile([C, RPC + 2, W + 2], F32)
            nc.gpsimd.memset(racc[:], 0.0)

            def fold(j, matched):
                """Transpose matched patches to channel-major and overlap-add."""
                matched3 = matched[:].rearrange("p (a b) -> p a b", b=9)
                n0 = j * 128
                r0, c0 = n0 // W, n0 % W
                seg1 = (r0, c0, W - c0, 0)
                seg2 = (r0 + 1, 0, 128 - (W - c0), W - c0)
                trT = psT.tile([128, 9, 128], F32, tag="psT", name=f"trT_{j}")
                for k in range(9):
                    nc.tensor.transpose(trT[:, k], matched3[:, :, k], ident[:])
                for k in range(9):
                    ki, kj = k // 3, k % 3
                    for (r, c, ln, off) in (seg1, seg2):
                        nc.vector.tensor_add(
                            racc[:, r + ki, c + kj : c + kj + ln],
                            racc[:, r + ki, c + kj : c + kj + ln],
                            trT[:, k, off : off + ln],
                        )

            DR = mybir.MatmulPerfMode.DoubleRow
            deferred = None
            for j in range(NT):
                S_sb = bigp.tile([128, M], BF16, tag="S_sb", bufs=2)

                for g in range(NG):
                    pt = psS.tile([128, MG], F32, tag="psS", name=f"ps_{j}_{g}")
                    for p in range(KC // 2):
                        nc.tensor.matmul(
                            out=pt[:],
                            lhsT=cvT_sb[:, 2 * p : 2 * p + 2, ts(j, 128)],
                            rhs=svn_sb[:, g, 2 * p : 2 * p + 2, :],
                            start=(p == 0),
                            stop=(p == KC // 2 - 1),
                            perf_mode=DR,
                        )
                    # PSUM -> SBUF on the Scalar engine (keeps DVE free)
                    nc.scalar.copy(S_sb[:, ts(g, MG)], pt[:])

                smax = workp.tile([128, 1], BF16, tag="smax")
                nc.vector.tensor_reduce(
                    out=smax[:],
                    in_=S_sb[:],
                    axis=mybir.AxisListType.X,
                    op=mybir.AluOpType.max,
                )
                idx8 = workp.tile([128, 8], U32, tag="idx8")
                nc.vector.max_index(
                    idx8[:], smax[:, 0:1].to_broadcast((128, 8)), S_sb[:]
                )
                bestu = workp.tile([128, 1], U32, tag="bestu")
                nc.vector.tensor_copy(bestu[:], idx8[:, 0:1])
                nc.sync.dma_start(out=idx_out[j], in_=bestu[:])

                # gather matched style patch rows (n-major); the indirect
                # DMA needs a flat 2D dest (3D dest tiles fetch garbage)
                matched = workp.tile([128, D], F32, tag="matched")
                nc.gpsimd.indirect_dma_start(
                    out=matched[:],
                    out_offset=None,
                    in_=sprows[:],
                    in_offset=IndirectOffsetOnAxis(ap=bestu[:, 0:1], axis=0),
                )

                # fold of the previous tile, deferred so tile j+1's matmuls
                # are already queued on the PE before these transposes
                if deferred is not None:
                    fold(*deferred)
                deferred = (j, matched)

            fold(*deferred)
            nc.sync.dma_start(out=racc_out[:], in_=racc[:, :, 1 : 1 + W])

    if not nc.is_finalized():
        nc.finalize()
    return nc


_PROGRAM = None


def _get_program():
    global _PROGRAM
    if _PROGRAM is None:
        _PROGRAM = build_program()
    return _PROGRAM


def _patch_rows(x):
    """(C, R, Cc) padded map -> ((R-2)*(Cc-2), C*9) patch rows, (c,ki,kj)."""
    w = np.lib.stride_tricks.sliding_window_view(x, (PW, PW), axis=(1, 2))
    return np.ascontiguousarray(
        w.transpose(1, 2, 0, 3, 4).reshape((x.shape[1] - 2) * (x.shape[2] - 2), -1)
    )


def _host_prep(content_feats, style_feats):
    """Build per-core input maps."""
    f8 = mybir.dt.np(FP8)
    cf = np.ascontiguousarray(np.asarray(content_feats, dtype=np.float32)[0])
    sf = np.ascontiguousarray(np.asarray(style_feats, dtype=np.float32)[0])
    cpad = np.pad(cf, ((0, 0), (1, 1), (1, 1)))
    spad = np.pad(sf, ((0, 0), (1, 1), (1, 1)))
    sprows = _patch_rows(spad)
    invn = 1.0 / np.maximum(
        np.linalg.norm(sprows, axis=1), np.float32(1e-12)
    ).astype(np.float32)
    svn8 = (sprows * invn[:, None]).astype(f8)
    # (M, D) -> (NG, 128, KC, MG): svnT[g, d, c, m] = svn[g*MG+m, c*128+d]
    svnT = np.zeros((NG, 128, KC, MG), dtype=f8)
    svnT[:, :, : D // 128, :] = (
        svn8.reshape(NG, MG, D // 128, 128).transpose(0, 3, 2, 1)
    )
    svnT = np.ascontiguousarray(svnT)
    in_maps = []
    for i in range(NCORES):
        cslab = np.ascontiguousarray(cpad[:, i * RPC : i * RPC + RPC + 2, :])
        cv8 = _patch_rows(cslab).astype(f8)      # (NSH, D)
        cvT = np.zeros((128, KC, NSH), dtype=f8)
        cvT[:, : D // 128, :] = cv8.reshape(NSH, D // 128, 128).transpose(2, 1, 0)
        in_maps.append(
            {
                "cvT8": np.ascontiguousarray(cvT),
                "svnT8": svnT,
                "sprows": sprows,
            }
        )
    return cf, in_maps


_DIVISOR = None


def _fold_divisor():
    global _DIVISOR
    if _DIVISOR is None:
        cnt = np.full(H, 3, dtype=np.float32)
        cnt[0] = cnt[-1] = 2
        _DIVISOR = np.outer(cnt, cnt).astype(np.float32) + np.float32(1e-8)
    return _DIVISOR


def _host_combine(cf, results):
    acc = np.zeros((C, H + 2, W), dtype=np.float32)
    for i in range(NCORES):
        acc[:, i * RPC : i * RPC + RPC + 2, :] += results[i]["racc_out"]
    recon = acc[:, 1 : 1 + H, :] / _fold_divisor()[None, :, :]
    diff = cf - recon
    return np.float32(np.mean(np.square(diff), dtype=np.float64))


def run(content_feats, style_feats, trace=False):
    nc = _get_program()
    cf, in_maps = _host_prep(content_feats, style_feats)
    res = run_bass_kernel_spmd(
        nc, in_maps, core_ids=list(range(NCORES)), trace=trace
    )
    mse = _host_combine(cf, res.results)
    return mse, res


def kernel(content_feats, style_feats):
    mse, _ = run(content_feats, style_feats)
    return np.array(mse, dtype=np.float32)


# revision 12
# speedup vs baseline: 2.3707x; 1.1617x over previous
"""CNN-MRF loss (retrieval kNN) on 8 Trainium2 NeuronCores.

Reference: cosine-similarity argmax between all 96x96 content patches and
96x96 style patches (3x3xC=128 patches, d=1152), gather matched style
patches, fold (overlap-add), MSE against content features.

Sharding: content-patch axis N split 8 ways (12 grid rows / core), style
replicated.  Per core, per 128-patch tile j:
  similarity: fp8(e4m3) matmul of content patch rows against
     HOST-PRE-NORMALIZED style patch rows (style side absorbs 1/||s||, so
     no on-device scaling pass is needed).  Contraction over d=1152 runs
     as 5 DoubleRow passes (2x fp8 rate, 256-deep each, zero-padded to 10
     chunks of 128).  PSUM -> SBUF (bf16) copies run on the otherwise-idle
     Scalar engine, so PSUM drain never waits on the DVE.
     fp8 scoring moves the argmax for ~6% of patches to a near-equal
     neighbour; measured end-MSE error 1.3e-4, far inside tolerance.
  argmax (two-level): ONE DVE scan produces all 18 per-group maxima
     (tensor_reduce axis=X over S viewed [128,18,512]); a tiny argmax over
     the 18 picks each partition's winning group; S is mirrored to a DRAM
     scratch tile so an indirect DMA can fetch each partition's winning
     512-wide group; a short max_index over 512 yields the final index.
     This avoids a second full 9216-element DVE scan.
  fold-by-matmul: matched (bf16) style rows gathered by indirect DMA are
     folded via 9 PSUM-accumulated matmuls against constant 0/1 scatter
     matrices (out[c,p] = sum_n matched[n,c,k] * A[n,k,p]), replacing 9 PE
     transposes + 18 DVE adds with one DVE add of a [128,392] strip.
     Deferred one iteration so the PE never waits on the argmax chain.
Host: sums the 8 overlapping strips, divides by fold counts, MSE.
"""
import os
import sys
import numpy as np

for _p in ("/opt/trn_rl_repo",):
    if _p not in sys.path:
        sys.path.insert(0, _p)

import concourse.bass as bass
import concourse.bacc as bacc
import concourse.mybir as mybir
from concourse.bass import IndirectOffsetOnAxis
from concourse.bass_utils import run_bass_kernel_spmd
from concourse.tile import TileContext
from concourse.masks import make_identity

F32 = mybir.dt.float32
BF16 = mybir.dt.bfloat16
FP8 = mybir.dt.float8e4
U32 = mybir.dt.uint32

C = 128          # channels
H = W = 96       # feature-map spatial dims
PW = 3           # patch size
N = H * W        # content patches total (9216)
M = N            # style patches (9216)
D = C * PW * PW  # patch vector length (1152)
NCORES = 8
RPC = H // NCORES       # content grid rows per core (12)
NSH = RPC * W           # content patches per core (1152)
NT = NSH // 128         # n-tiles of 128 per core (9)
MG = 512                # style patches per matmul group
NG = M // MG            # matmul groups (18)
KC = 10                 # contraction chunks of 128 (9 real + 1 zero pad)
PSTRIP = 4 * (W + 2)    # fold output strip length (4 rows x 98)

TWOLVL = os.environ.get("TWOLVL", "1") == "1"
FOLDMM = os.environ.get("FOLDMM", "1") == "1"


def ts(i, size):
    return slice(i * size, (i + 1) * size)


def build_program():
    nc = bacc.Bacc()

    cvT8 = nc.declare_dram_parameter("cvT8", [128, KC, NSH], FP8, isOutput=False)
    svnT8 = nc.declare_dram_parameter("svnT8", [NG, 128, KC, MG], FP8, isOutput=False)
    sprows = nc.declare_dram_parameter("sprows", [M, D], BF16, isOutput=False)
    foldA = nc.declare_dram_parameter("foldA", [128, 3, 9, PSTRIP], BF16,
                                      isOutput=False)
    nrow18 = nc.declare_dram_parameter("nrow18", [128, 1], F32, isOutput=False)
    idx_out = nc.declare_dram_parameter("idx_out", [NT, 128, 1], U32, isOutput=True)
    racc_out = nc.declare_dram_parameter(
        "racc_out", [C, RPC + 2, W], F32, isOutput=True
    )

    with TileContext(nc) as tc:
        with (
            tc.tile_pool(name="const", bufs=1) as constp,
            tc.tile_pool(name="big", bufs=1) as bigp,
            tc.tile_pool(name="work", bufs=2) as workp,
            tc.tile_pool(name="dram", bufs=1, space="DRAM") as dramp,
            tc.tile_pool(name="psS", bufs=4, space="PSUM") as psS,
            tc.tile_pool(name="psF", bufs=2, space="PSUM") as psF,
        ):
            # ---- loads (spread across engine queues so the DMA triggers
            # don't serialize on one queue) ----
            cvT_sb = bigp.tile([128, KC, NSH], FP8)
            svn_sb = bigp.tile([128, NG, KC, MG], FP8)
            qs = [nc.sync, nc.scalar, nc.gpsimd]
            nc.sync.dma_start(out=svn_sb[:, 0], in_=svnT8[0])
            nc.scalar.dma_start(out=cvT_sb[:], in_=cvT8[:])
            for g in range(1, NG):
                qs[g % 3].dma_start(out=svn_sb[:, g], in_=svnT8[g])
            A_sb = constp.tile([128, 3, 9, PSTRIP], BF16)
            nc.scalar.dma_start(out=A_sb[:], in_=foldA[:])
            nrow_sb = constp.tile([128, 1], F32)
            nc.sync.dma_start(out=nrow_sb[:], in_=nrow18[:])

            ident = None
            if not FOLDMM:
                ident = constp.tile([128, 128], F32)
                make_identity(nc, ident[:])

            racc = bigp.tile([C, (RPC + 2) * (W + 2)], F32)
            nc.gpsimd.memset(racc[:], 0.0)

            S_dram = dramp.tile([128 * NG, MG], BF16)

            def fold(j, matched):
                """Fold matched patches into racc via scatter matmuls."""
                matched3 = matched[:].rearrange("p (a b) -> p a b", b=9)
                n0 = j * 128
                r0, c0 = n0 // W, n0 % W
                if FOLDMM:
                    pf = psF.tile([128, PSTRIP], F32, tag="psF", name=f"pf_{j}")
                    for k in range(9):
                        nc.tensor.matmul(
                            out=pf[:],
                            lhsT=matched3[:, :, k],
                            rhs=A_sb[:, c0 // 32, k, :],
                            start=(k == 0),
                            stop=(k == 8),
                        )
                    nc.vector.tensor_add(
                        racc[:, r0 * (W + 2) : r0 * (W + 2) + PSTRIP],
                        racc[:, r0 * (W + 2) : r0 * (W + 2) + PSTRIP],
                        pf[:],
                    )
                else:
                    racc3 = racc[:].rearrange("p (a b) -> p a b", b=W + 2)
                    seg1 = (r0, c0, W - c0, 0)
                    seg2 = (r0 + 1, 0, 128 - (W - c0), W - c0)
                    trT = psF.tile([128, 9, 128], F32, tag="psT", name=f"trT_{j}")
                    for k in range(9):
                        nc.tensor.transpose(trT[:, k], matched3[:, :, k], ident[:])
                    for k in range(9):
                        ki, kj = k // 3, k % 3
                        for (r, c, ln, off) in (seg1, seg2):
                            nc.vector.tensor_add(
                                racc3[:, r + ki, c + kj : c + kj + ln],
                                racc3[:, r + ki, c + kj : c + kj + ln],
                                trT[:, k, off : off + ln],
                            )

            DR = mybir.MatmulPerfMode.DoubleRow
            deferred = None
            for j in range(NT):
                S_sb = bigp.tile([128, NG, MG], BF16, tag="S_sb", bufs=2)

                for g in range(NG):
                    pt = psS.tile([128, MG], F32, tag="psS", name=f"ps_{j}_{g}")
                    for p in range(KC // 2):
                        nc.tensor.matmul(
                            out=pt[:],
                            lhsT=cvT_sb[:, 2 * p : 2 * p + 2, ts(j, 128)],
                            rhs=svn_sb[:, g, 2 * p : 2 * p + 2, :],
                            start=(p == 0),
                            stop=(p == KC // 2 - 1),
                            perf_mode=DR,
                        )
                    # PSUM -> SBUF on the Scalar engine (keeps DVE free)
                    nc.scalar.copy(S_sb[:, g], pt[:])

                # all 18 group maxima from ONE scan
                gmax = workp.tile([128, NG], BF16, tag="gmax")
                nc.vector.tensor_reduce(
                    out=gmax[:],
                    in_=S_sb[:],
                    axis=mybir.AxisListType.X,
                    op=mybir.AluOpType.max,
                )
                vm8 = workp.tile([128, 8], BF16, tag="vm8")
                nc.vector.max(vm8[:], gmax[:])

                if TWOLVL:
                    # mirror S to DRAM so an indirect DMA can fetch each
                    # partition's winning group
                    nc.sync.dma_start(
                        out=S_dram[:].rearrange("(p g) m -> p g m", g=NG),
                        in_=S_sb[:],
                    )
                    gi8 = workp.tile([128, 8], U32, tag="gi8")
                    nc.vector.max_index(gi8[:], vm8[:], gmax[:])
                    g8f = workp.tile([128, 1], F32, tag="g8f")
                    nc.vector.tensor_copy(g8f[:], gi8[:, 0:1])
                    offf = workp.tile([128, 1], F32, tag="offf")
                    nc.vector.scalar_tensor_tensor(
                        out=offf[:],
                        in0=g8f[:],
                        scalar=0.0,
                        in1=nrow_sb[:],
                        op0=mybir.AluOpType.add,
                        op1=mybir.AluOpType.add,
                    )
                    offu = workp.tile([128, 1], U32, tag="offu")
                    nc.vector.tensor_copy(offu[:], offf[:])
                    Sg = workp.tile([128, MG], BF16, tag="Sg")
                    nc.gpsimd.indirect_dma_start(
                        out=Sg[:],
                        out_offset=None,
                        in_=S_dram[:],
                        in_offset=IndirectOffsetOnAxis(ap=offu[:, 0:1], axis=0),
                    )
                    li8 = workp.tile([128, 8], U32, tag="li8")
                    vm8b = workp.tile([128, 8], BF16, tag="vm8b")
                    nc.vector.tensor_copy(
                        vm8b[:], vm8[:, 0:1].to_broadcast((128, 8))
                    )
                    nc.vector.max_index(li8[:], vm8b[:], Sg[:])
                    lf = workp.tile([128, 1], F32, tag="lf")
                    nc.vector.tensor_copy(lf[:], li8[:, 0:1])
                    bestf = workp.tile([128, 1], F32, tag="bestf")
                    nc.vector.scalar_tensor_tensor(
                        out=bestf[:],
                        in0=g8f[:],
                        scalar=float(MG),
                        in1=lf[:],
                        op0=mybir.AluOpType.mult,
                        op1=mybir.AluOpType.add,
                    )
                    bestu = workp.tile([128, 1], U32, tag="bestu")
                    nc.vector.tensor_copy(bestu[:], bestf[:])
                else:
                    idx8 = workp.tile([128, 8], U32, tag="idx8")
                    nc.vector.max_index(
                        idx8[:], vm8[:, 0:1].to_broadcast((128, 8)), S_sb[:]
                    )
                    bestu = workp.tile([128, 1], U32, tag="bestu")
                    nc.vector.tensor_copy(bestu[:], idx8[:, 0:1])

                nc.sync.dma_start(out=idx_out[j], in_=bestu[:])

                # gather matched style patch rows (n-major); the indirect
                # DMA needs a flat 2D dest (3D dest tiles fetch garbage)
                matched = workp.tile([128, D], BF16, tag="matched")
                nc.gpsimd.indirect_dma_start(
                    out=matched[:],
                    out_offset=None,
                    in_=sprows[:],
                    in_offset=IndirectOffsetOnAxis(ap=bestu[:, 0:1], axis=0),
                )

                # fold of the previous tile, deferred so tile j+1's matmuls
                # are already queued on the PE before these matmuls
                if deferred is not None:
                    fold(*deferred)
                deferred = (j, matched)

            fold(*deferred)
            racc3 = racc[:].rearrange("p (a b) -> p a b", b=W + 2)
            nc.sync.dma_start(out=racc_out[:], in_=racc3[:, :, 1 : 1 + W])

    if not nc.is_finalized():
        nc.finalize()
    return nc


_PROGRAM = None


def _get_program():
    global _PROGRAM
    if _PROGRAM is None:
        _PROGRAM = build_program()
    return _PROGRAM


def _patch_rows(x):
    """(C, R, Cc) padded map -> ((R-2)*(Cc-2), C*9) patch rows, (c,ki,kj)."""
    w = np.lib.stride_tricks.sliding_window_view(x, (PW, PW), axis=(1, 2))
    return np.ascontiguousarray(
        w.transpose(1, 2, 0, 3, 4).reshape((x.shape[1] - 2) * (x.shape[2] - 2), -1)
    )


_FOLD_A = None


def _fold_A():
    """(128, 3, 9, PSTRIP) 0/1 scatter matrices for fold-by-matmul."""
    global _FOLD_A
    if _FOLD_A is None:
        bf = mybir.dt.np(BF16)
        A = np.zeros((128, 3, 9, PSTRIP), dtype=bf)
        for v in range(3):
            c0 = 32 * v
            for n in range(128):
                rr, cc = (c0 + n) // W, (c0 + n) % W
                for k in range(9):
                    ki, kj = k // 3, k % 3
                    A[n, v, k, (rr + ki) * (W + 2) + cc + kj] = 1.0
        _FOLD_A = A
    return _FOLD_A


def _host_prep(content_feats, style_feats):
    """Build per-core input maps."""
    f8 = mybir.dt.np(FP8)
    bf = mybir.dt.np(BF16)
    cf = np.ascontiguousarray(np.asarray(content_feats, dtype=np.float32)[0])
    sf = np.ascontiguousarray(np.asarray(style_feats, dtype=np.float32)[0])
    cpad = np.pad(cf, ((0, 0), (1, 1), (1, 1)))
    spad = np.pad(sf, ((0, 0), (1, 1), (1, 1)))
    sprows = _patch_rows(spad)
    invn = 1.0 / np.maximum(
        np.linalg.norm(sprows, axis=1), np.float32(1e-12)
    ).astype(np.float32)
    svn8 = (sprows * invn[:, None]).astype(f8)
    # (M, D) -> (NG, 128, KC, MG): svnT[g, d, c, m] = svn[g*MG+m, c*128+d]
    svnT = np.zeros((NG, 128, KC, MG), dtype=f8)
    svnT[:, :, : D // 128, :] = (
        svn8.reshape(NG, MG, D // 128, 128).transpose(0, 3, 2, 1)
    )
    svnT = np.ascontiguousarray(svnT)
    sprows_bf = sprows.astype(bf)
    nrow = (np.arange(128, dtype=np.float32) * NG).reshape(128, 1)
    foldA = _fold_A()
    in_maps = []
    for i in range(NCORES):
        cslab = np.ascontiguousarray(cpad[:, i * RPC : i * RPC + RPC + 2, :])
        cv8 = _patch_rows(cslab).astype(f8)      # (NSH, D)
        cvT = np.zeros((128, KC, NSH), dtype=f8)
        cvT[:, : D // 128, :] = cv8.reshape(NSH, D // 128, 128).transpose(2, 1, 0)
        in_maps.append(
            {
                "cvT8": np.ascontiguousarray(cvT),
                "svnT8": svnT,
                "sprows": sprows_bf,
                "foldA": foldA,
                "nrow18": nrow,
            }
        )
    return cf, in_maps


_DIVISOR = None


def _fold_divisor():
    global _DIVISOR
    if _DIVISOR is None:
        cnt = np.full(H, 3, dtype=np.float32)
        cnt[0] = cnt[-1] = 2
        _DIVISOR = np.outer(cnt, cnt).astype(np.float32) + np.float32(1e-8)
    return _DIVISOR


def _host_combine(cf, results):
    acc = np.zeros((C, H + 2, W), dtype=np.float32)
    for i in range(NCORES):
        acc[:, i * RPC : i * RPC + RPC + 2, :] += results[i]["racc_out"]
    recon = acc[:, 1 : 1 + H, :] / _fold_divisor()[None, :, :]
    diff = cf - recon
    return np.float32(np.mean(np.square(diff), dtype=np.float64))


def run(content_feats, style_feats, trace=False):
    nc = _get_program()
    cf, in_maps = _host_prep(content_feats, style_feats)
    res = run_bass_kernel_spmd(
        nc, in_maps, core_ids=list(range(NCORES)), trace=trace
    )
    mse = _host_combine(cf, res.results)
    return mse, res


def kernel(content_feats, style_feats):
    mse, _ = run(content_feats, style_feats)
    return np.array(mse, dtype=np.float32)


# revision 15
# speedup vs baseline: 2.3826x; 1.0050x over previous
"""CNN-MRF loss (retrieval kNN) on 8 Trainium2 NeuronCores.

Reference: cosine-similarity argmax between all 96x96 content patches and
96x96 style patches (3x3xC=128 patches, d=1152), gather matched style
patches, fold (overlap-add), MSE against content features.

Sharding: content-patch axis N split 8 ways (12 grid rows / core), style
replicated.  Per core, per 128-patch tile j:
  similarity: fp8(e4m3) matmul of content patch rows against
     HOST-PRE-NORMALIZED style patch rows (style side absorbs 1/||s||, so
     no on-device scaling pass is needed).  Contraction over d=1152 runs
     as 5 DoubleRow passes (2x fp8 rate, 256-deep each, zero-padded to 10
     chunks of 128).  PSUM -> SBUF (bf16) copies run on the otherwise-idle
     Scalar engine, so PSUM drain never waits on the DVE.
     fp8 scoring moves the argmax for ~6% of patches to a near-equal
     neighbour; measured end-MSE error 1.3e-4, far inside tolerance.
  argmax (two-level): ONE DVE scan produces all 18 per-group maxima
     (tensor_reduce axis=X over S viewed [128,18,512]); a tiny argmax over
     the 18 picks each partition's winning group; S is mirrored to a DRAM
     scratch tile so an indirect DMA can fetch each partition's winning
     512-wide group; a short max_index over 512 yields the final index.
     This avoids a second full 9216-element DVE scan.
  fold-by-matmul: matched (bf16) style rows gathered by indirect DMA are
     folded via 9 PSUM-accumulated matmuls against constant 0/1 scatter
     matrices (out[c,p] = sum_n matched[n,c,k] * A[n,k,p]), replacing 9 PE
     transposes + 18 DVE adds with one DVE add of a [128,392] strip.
     Deferred one iteration so the PE never waits on the argmax chain.
Host: sums the 8 overlapping strips, divides by fold counts, MSE.
"""
import os
import sys
import numpy as np

for _p in ("/opt/trn_rl_repo",):
    if _p not in sys.path:
        sys.path.insert(0, _p)

import concourse.bass as bass
import concourse.bacc as bacc
import concourse.mybir as mybir
from concourse.bass import IndirectOffsetOnAxis
from concourse.bass_utils import run_bass_kernel_spmd
from concourse.tile import TileContext
from concourse.masks import make_identity

F32 = mybir.dt.float32
BF16 = mybir.dt.bfloat16
FP8 = mybir.dt.float8e4
U32 = mybir.dt.uint32

C = 128          # channels
H = W = 96       # feature-map spatial dims
PW = 3           # patch size
N = H * W        # content patches total (9216)
M = N            # style patches (9216)
D = C * PW * PW  # patch vector length (1152)
NCORES = 8
RPC = H // NCORES       # content grid rows per core (12)
NSH = RPC * W           # content patches per core (1152)
NT = NSH // 128         # n-tiles of 128 per core (9)
MG = 512                # style patches per matmul group
NG = M // MG            # matmul groups (18)
KC = 10                 # contraction chunks of 128 (9 real + 1 zero pad)
PSTRIP = 4 * (W + 2)    # fold output strip length (4 rows x 98)

TWOLVL = os.environ.get("TWOLVL", "1") == "1"
FOLDMM = os.environ.get("FOLDMM", "1") == "1"


def ts(i, size):
    return slice(i * size, (i + 1) * size)


def build_program():
    nc = bacc.Bacc()

    cvT8 = nc.declare_dram_parameter("cvT8", [128, KC, NSH], FP8, isOutput=False)
    svnT8 = nc.declare_dram_parameter("svnT8", [NG, 128, KC, MG], FP8, isOutput=False)
    sprows = nc.declare_dram_parameter("sprows", [M, D], BF16, isOutput=False)
    foldA = nc.declare_dram_parameter("foldA", [128, 3, 9, PSTRIP], BF16,
                                      isOutput=False)
    nrow18 = nc.declare_dram_parameter("nrow18", [128, 1], F32, isOutput=False)
    idx_out = nc.declare_dram_parameter("idx_out", [NT, 128, 1], U32, isOutput=True)
    racc_out = nc.declare_dram_parameter(
        "racc_out", [C, RPC + 2, W], F32, isOutput=True
    )

    with TileContext(nc) as tc:
        with (
            tc.tile_pool(name="const", bufs=1) as constp,
            tc.tile_pool(name="big", bufs=1) as bigp,
            tc.tile_pool(name="work", bufs=2) as workp,
            tc.tile_pool(name="dram", bufs=1, space="DRAM") as dramp,
            tc.tile_pool(name="psS", bufs=4, space="PSUM") as psS,
            tc.tile_pool(name="psF", bufs=2, space="PSUM") as psF,
        ):
            # ---- loads (spread across engine queues so the DMA triggers
            # don't serialize on one queue) ----
            cvT_sb = bigp.tile([128, KC, NSH], FP8)
            svn_sb = bigp.tile([128, NG, KC, MG], FP8)
            qs = [nc.sync, nc.scalar, nc.gpsimd]
            nc.scalar.dma_start(out=cvT_sb[:, 0:2], in_=cvT8[:, 0:2])
            nc.sync.dma_start(out=svn_sb[:, 0], in_=svnT8[0])
            nc.scalar.dma_start(out=cvT_sb[:, 2:KC], in_=cvT8[:, 2:KC])
            for g in range(1, NG):
                qs[g % 3].dma_start(out=svn_sb[:, g], in_=svnT8[g])
            A_sb = constp.tile([128, 3, 9, PSTRIP], BF16)
            nc.scalar.dma_start(out=A_sb[:], in_=foldA[:])
            nrow_sb = constp.tile([128, 1], F32)
            nc.sync.dma_start(out=nrow_sb[:], in_=nrow18[:])

            ident = None
            if not FOLDMM:
                ident = constp.tile([128, 128], F32)
                make_identity(nc, ident[:])

            racc = bigp.tile([C, (RPC + 2) * (W + 2)], F32)
            nc.gpsimd.memset(racc[:], 0.0)

            S_dram = dramp.tile([128 * NG, MG], BF16)

            def fold(j, matched):
                """Fold matched patches into racc via scatter matmuls."""
                matched3 = matched[:].rearrange("p (a b) -> p a b", b=9)
                n0 = j * 128
                r0, c0 = n0 // W, n0 % W
                if FOLDMM:
                    pf = psF.tile([128, PSTRIP], F32, tag="psF", name=f"pf_{j}")
                    for k in range(9):
                        nc.tensor.matmul(
                            out=pf[:],
                            lhsT=matched3[:, :, k],
                            rhs=A_sb[:, c0 // 32, k, :],
                            start=(k == 0),
                            stop=(k == 8),
                        )
                    nc.vector.tensor_add(
                        racc[:, r0 * (W + 2) : r0 * (W + 2) + PSTRIP],
                        racc[:, r0 * (W + 2) : r0 * (W + 2) + PSTRIP],
                        pf[:],
                    )
                else:
                    racc3 = racc[:].rearrange("p (a b) -> p a b", b=W + 2)
                    seg1 = (r0, c0, W - c0, 0)
                    seg2 = (r0 + 1, 0, 128 - (W - c0), W - c0)
                    trT = psF.tile([128, 9, 128], F32, tag="psT", name=f"trT_{j}")
                    for k in range(9):
                        nc.tensor.transpose(trT[:, k], matched3[:, :, k], ident[:])
                    for k in range(9):
                        ki, kj = k // 3, k % 3
                        for (r, c, ln, off) in (seg1, seg2):
                            nc.vector.tensor_add(
                                racc3[:, r + ki, c + kj : c + kj + ln],
                                racc3[:, r + ki, c + kj : c + kj + ln],
                                trT[:, k, off : off + ln],
                            )

            DR = mybir.MatmulPerfMode.DoubleRow
            deferred = None
            for j in range(NT):
                S_sb = bigp.tile([128, NG, MG], BF16, tag="S_sb", bufs=2)

                for g in range(NG):
                    pt = psS.tile([128, MG], F32, tag="psS", name=f"ps_{j}_{g}")
                    for p in range(KC // 2):
                        nc.tensor.matmul(
                            out=pt[:],
                            lhsT=cvT_sb[:, 2 * p : 2 * p + 2, ts(j, 128)],
                            rhs=svn_sb[:, g, 2 * p : 2 * p + 2, :],
                            start=(p == 0),
                            stop=(p == KC // 2 - 1),
                            perf_mode=DR,
                        )
                    # PSUM -> SBUF on the Scalar engine (keeps DVE free)
                    nc.scalar.copy(S_sb[:, g], pt[:])
                    if TWOLVL and g % 3 == 2:
                        # eager S mirror to DRAM, 3 groups at a time, so the
                        # winning-group gather never waits on a bulk DMA
                        nc.sync.dma_start(
                            out=S_dram[:].rearrange("(p g) m -> p g m", g=NG)[
                                :, g - 2 : g + 1
                            ],
                            in_=S_sb[:, g - 2 : g + 1],
                        )

                # all 18 group maxima from ONE scan
                gmax = workp.tile([128, NG], BF16, tag="gmax")
                nc.vector.tensor_reduce(
                    out=gmax[:],
                    in_=S_sb[:],
                    axis=mybir.AxisListType.X,
                    op=mybir.AluOpType.max,
                )
                vm8 = workp.tile([128, 8], BF16, tag="vm8")
                nc.vector.max(vm8[:], gmax[:])

                if TWOLVL:
                    gi8 = workp.tile([128, 8], U32, tag="gi8")
                    nc.vector.max_index(gi8[:], vm8[:], gmax[:])
                    g8f = workp.tile([128, 1], F32, tag="g8f")
                    nc.vector.tensor_copy(g8f[:], gi8[:, 0:1])
                    offf = workp.tile([128, 1], F32, tag="offf")
                    nc.vector.scalar_tensor_tensor(
                        out=offf[:],
                        in0=g8f[:],
                        scalar=0.0,
                        in1=nrow_sb[:],
                        op0=mybir.AluOpType.add,
                        op1=mybir.AluOpType.add,
                    )
                    offu = workp.tile([128, 1], U32, tag="offu")
                    nc.vector.tensor_copy(offu[:], offf[:])
                    Sg = workp.tile([128, MG], BF16, tag="Sg")
                    nc.gpsimd.indirect_dma_start(
                        out=Sg[:],
                        out_offset=None,
                        in_=S_dram[:],
                        in_offset=IndirectOffsetOnAxis(ap=offu[:, 0:1], axis=0),
                    )
                    li8 = workp.tile([128, 8], U32, tag="li8")
                    vm8b = workp.tile([128, 8], BF16, tag="vm8b")
                    nc.vector.tensor_copy(
                        vm8b[:], vm8[:, 0:1].to_broadcast((128, 8))
                    )
                    nc.vector.max_index(li8[:], vm8b[:], Sg[:])
                    lf = workp.tile([128, 1], F32, tag="lf")
                    nc.vector.tensor_copy(lf[:], li8[:, 0:1])
                    bestf = workp.tile([128, 1], F32, tag="bestf")
                    nc.vector.scalar_tensor_tensor(
                        out=bestf[:],
                        in0=g8f[:],
                        scalar=float(MG),
                        in1=lf[:],
                        op0=mybir.AluOpType.mult,
                        op1=mybir.AluOpType.add,
                    )
                    bestu = workp.tile([128, 1], U32, tag="bestu")
                    nc.vector.tensor_copy(bestu[:], bestf[:])
                else:
                    idx8 = workp.tile([128, 8], U32, tag="idx8")
                    nc.vector.max_index(
                        idx8[:], vm8[:, 0:1].to_broadcast((128, 8)), S_sb[:]
                    )
                    bestu = workp.tile([128, 1], U32, tag="bestu")
                    nc.vector.tensor_copy(bestu[:], idx8[:, 0:1])

                nc.sync.dma_start(out=idx_out[j], in_=bestu[:])

                # gather matched style patch rows (n-major); the indirect
                # DMA needs a flat 2D dest (3D dest tiles fetch garbage)
                matched = workp.tile([128, D], BF16, tag="matched")
                nc.gpsimd.indirect_dma_start(
                    out=matched[:],
                    out_offset=None,
                    in_=sprows[:],
                    in_offset=IndirectOffsetOnAxis(ap=bestu[:, 0:1], axis=0),
                )

                # fold of the previous tile, deferred so tile j+1's matmuls
                # are already queued on the PE before these matmuls
                if deferred is not None:
                    fold(*deferred)
                deferred = (j, matched)

            fold(*deferred)
            racc3 = racc[:].rearrange("p (a b) -> p a b", b=W + 2)
            nc.sync.dma_start(out=racc_out[:], in_=racc3[:, :, 1 : 1 + W])

    if not nc.is_finalized():
        nc.finalize()
    return nc


_PROGRAM = None


def _get_program():
    global _PROGRAM
    if _PROGRAM is None:
        _PROGRAM = build_program()
    return _PROGRAM


def _patch_rows(x):
    """(C, R, Cc) padded map -> ((R-2)*(Cc-2), C*9) patch rows, (c,ki,kj)."""
    w = np.lib.stride_tricks.sliding_window_view(x, (PW, PW), axis=(1, 2))
    return np.ascontiguousarray(
        w.transpose(1, 2, 0, 3, 4).reshape((x.shape[1] - 2) * (x.shape[2] - 2), -1)
    )


_FOLD_A = None


def _fold_A():
    """(128, 3, 9, PSTRIP) 0/1 scatter matrices for fold-by-matmul."""
    global _FOLD_A
    if _FOLD_A is None:
        bf = mybir.dt.np(BF16)
        A = np.zeros((128, 3, 9, PSTRIP), dtype=bf)
        for v in range(3):
            c0 = 32 * v
            for n in range(128):
                rr, cc = (c0 + n) // W, (c0 + n) % W
                for k in range(9):
                    ki, kj = k // 3, k % 3
                    A[n, v, k, (rr + ki) * (W + 2) + cc + kj] = 1.0
        _FOLD_A = A
    return _FOLD_A


def _host_prep(content_feats, style_feats):
    """Build per-core input maps."""
    f8 = mybir.dt.np(FP8)
    bf = mybir.dt.np(BF16)
    cf = np.ascontiguousarray(np.asarray(content_feats, dtype=np.float32)[0])
    sf = np.ascontiguousarray(np.asarray(style_feats, dtype=np.float32)[0])
    cpad = np.pad(cf, ((0, 0), (1, 1), (1, 1)))
    spad = np.pad(sf, ((0, 0), (1, 1), (1, 1)))
    sprows = _patch_rows(spad)
    invn = 1.0 / np.maximum(
        np.linalg.norm(sprows, axis=1), np.float32(1e-12)
    ).astype(np.float32)
    svn8 = (sprows * invn[:, None]).astype(f8)
    # (M, D) -> (NG, 128, KC, MG): svnT[g, d, c, m] = svn[g*MG+m, c*128+d]
    svnT = np.zeros((NG, 128, KC, MG), dtype=f8)
    svnT[:, :, : D // 128, :] = (
        svn8.reshape(NG, MG, D // 128, 128).transpose(0, 3, 2, 1)
    )
    svnT = np.ascontiguousarray(svnT)
    sprows_bf = sprows.astype(bf)
    nrow = (np.arange(128, dtype=np.float32) * NG).reshape(128, 1)
    foldA = _fold_A()
    in_maps = []
    for i in range(NCORES):
        cslab = np.ascontiguousarray(cpad[:, i * RPC : i * RPC + RPC + 2, :])
        cv8 = _patch_rows(cslab).astype(f8)      # (NSH, D)
        cvT = np.zeros((128, KC, NSH), dtype=f8)
        cvT[:, : D // 128, :] = cv8.reshape(NSH, D // 128, 128).transpose(2, 1, 0)
        in_maps.append(
            {
                "cvT8": np.ascontiguousarray(cvT),
                "svnT8": svnT,
                "sprows": sprows_bf,
                "foldA": foldA,
                "nrow18": nrow,
            }
        )
    return cf, in_maps


_DIVISOR = None


def _fold_divisor():
    global _DIVISOR
    if _DIVISOR is None:
        cnt = np.full(H, 3, dtype=np.float32)
        cnt[0] = cnt[-1] = 2
        _DIVISOR = np.outer(cnt, cnt).astype(np.float32) + np.float32(1e-8)
    return _DIVISOR


def _host_combine(cf, results):
    acc = np.zeros((C, H + 2, W), dtype=np.float32)
    for i in range(NCORES):
        acc[:, i * RPC : i * RPC + RPC + 2, :] += results[i]["racc_out"]
    recon = acc[:, 1 : 1 + H, :] / _fold_divisor()[None, :, :]
    diff = cf - recon
    return np.float32(np.mean(np.square(diff), dtype=np.float64))


def run(content_feats, style_feats, trace=False):
    nc = _get_program()
    cf, in_maps = _host_prep(content_feats, style_feats)
    res = run_bass_kernel_spmd(
        nc, in_maps, core_ids=list(range(NCORES)), trace=trace
    )
    mse = _host_combine(cf, res.results)
    return mse, res


def kernel(content_feats, style_feats):
    mse, _ = run(content_feats, style_feats)
    return np.array(mse, dtype=np.float32)
